# revision 112
# baseline (speedup 1.0000x reference)
"""Trainium2 Bass kernel for ConformalGQA (b=4, t=1024, d=2048, 32 Q heads /
8 KV heads, hd=64, RoPE, causal, scores = (q.k - |q|^2/2 - |k|^2/2)/sqrt(hd)).

Sharding: 8-way tensor-parallel over heads. Core c owns Q heads 4c..4c+3 and
KV head c (Wq/Wk/Wv column-sharded, Wo row-sharded). Each core emits a full
(4096, 2048) partial output; the host sums the 8 partials.

Key device-side structure (v2 — engine-balanced, PSUM-budgeted pipeline):
 - The -0.5|q|^2 term of the scores is constant per query row, so softmax
   cancels it: it is DROPPED entirely. Only the k-norm survives, folded into
   the per-partition bias of the Exp activation (bias = -|k|^2/16). Scores
   are then bounded by |q|^2/16 (~3, max ~10): exp stays well inside fp32.
 - Projections run in half-token passes: per 512-token half, three [128,512]
   PSUM accumulators (q-heads 0:2, q-heads 2:4, K|V) accumulate over the 16
   d-chunks. 1 PSUM bank each; rope consumes them via an Act-engine copy to
   SBUF, so PE never waits on DVE.
 - Scores are computed as S^T (k on partitions, q on free dim) in >=256-wide
   pieces (remainder-free splits) to keep fp32r at 1 cycle/row.
 - P^T = exp(S^T/8 + bias) -> attnV accumulates [V | 1]^T @ P^T so the
   softmax denominator rides in row 64 of yhp for free.
 - PSUM budget (8 banks): proj pool 3x1, scores/misc pool 2x1, yhp 1x2,
   Wo out pool 1x1.
 - Engine split: PE matmuls; Act: exp + PSUM->SBUF staging copies; DVE:
   rope arithmetic + normalize; Pool(GpSimd): causal diag masks + out-DMA
   queue; SP: x-in DMA queue.
All matmuls run in float32r (single-pass fp32 PE mode).
"""

import sys

for _p in ("/opt/trn_rl_repo",):
    if _p not in sys.path:
        sys.path.insert(0, _p)

import numpy as np
import ml_dtypes
from contextlib import ExitStack

_BF = ml_dtypes.bfloat16

import concourse.bass as bass
import concourse.mybir as mybir
import concourse.tile as tile
from concourse.tile import add_dep_helper
from concourse import bacc
from concourse.bass_utils import run_bass_kernel_spmd

F32R = mybir.dt.float32r
F32 = mybir.dt.float32
BF16 = mybir.dt.bfloat16
AF = mybir.ActivationFunctionType

B, T, D, KV = 4, 1024, 2048, 512
H, HKV, HD = 32, 8, 64
P = 128
NCORES = 8
HPC = H // NCORES          # 4 q heads per core
DOUT = HPC * HD            # 256 q-proj cols per core
NDC = D // P               # 16 contraction chunks
NTC = T // P               # 8 key chunks per batch
ROPE_BASE = 10000.0

_COMPILED = {}


def _score_pieces(kc):
    """Column pieces [c0, c0+cw) covering queries [q0, T) with cw >= 256
    where possible (each piece is one matmul into its own 1-bank tile)."""
    q0 = kc * P
    w = T - q0
    if w <= 512:
        return [(q0, w)]
    if w - 512 >= 256:
        return [(q0, 512), (q0 + 512, w - 512)]
    return [(q0, w - 256), (q0 + w - 256, 256)]


def _attnv_pieces(kc):
    """Accumulation pieces into yhp [65, T]: must not cross the tile's
    internal PSUM bank boundary at column 512."""
    q0 = kc * P
    if q0 < 512:
        return [(q0, 512 - q0), (512, 512)]
    return [(q0, T - q0)]


def _build_nc():
    nc = bacc.Bacc("TRN2", target_bir_lowering=False, debug=False,
                   num_devices=NCORES)

    xT = nc.dram_tensor("xT", [D, B * T], F32R, kind="ExternalInput")
    wq = nc.dram_tensor("wq", [P, NDC, DOUT], F32R, kind="ExternalInput")
    wkv = nc.dram_tensor("wkv", [P, NDC, 2 * HD], F32R, kind="ExternalInput")
    wo = nc.dram_tensor("wo", [P, 2, D], F32R, kind="ExternalInput")
    cc = nc.dram_tensor("cc", [P, T], F32, kind="ExternalInput")
    ss = nc.dram_tensor("ss", [P, T], F32, kind="ExternalInput")
    o64n = nc.dram_tensor("o64n", [64, 2], F32R, kind="ExternalInput")
    o1x64 = nc.dram_tensor("o1x64", [1, 64], F32R, kind="ExternalInput")
    ident = nc.dram_tensor("ident", [64, 64], F32R, kind="ExternalInput")
    triu = nc.dram_tensor("triu", [P, P], BF16, kind="ExternalInput")
    triu2 = nc.dram_tensor("triu2", [P, 2 * P], BF16, kind="ExternalInput")
    onestc = nc.dram_tensor("onestc", [P, NTC], BF16, kind="ExternalInput")
    out = nc.dram_tensor("out", [B * T, D], BF16, kind="ExternalOutput")

    with tile.TileContext(nc) as tc:
        with ExitStack() as ctx:
            cpool = ctx.enter_context(tc.tile_pool(name="consts", bufs=1))
            wpool = ctx.enter_context(tc.tile_pool(name="weights", bufs=1))
            xpool = ctx.enter_context(tc.tile_pool(name="x", bufs=10))
            rawp = ctx.enter_context(tc.tile_pool(name="raw", bufs=4))
            tpool = ctx.enter_context(tc.tile_pool(name="tscr", bufs=4))
            qpool = ctx.enter_context(tc.tile_pool(name="qhat", bufs=4))
            kpool = ctx.enter_context(tc.tile_pool(name="khat", bufs=2))
            vpool = ctx.enter_context(tc.tile_pool(name="vhat", bufs=2))
            bpool = ctx.enter_context(tc.tile_pool(name="kbias", bufs=2))
            ptpool = ctx.enter_context(tc.tile_pool(name="pt", bufs=7))
            ypool = ctx.enter_context(tc.tile_pool(name="ysb", bufs=2))
            rpool = ctx.enter_context(tc.tile_pool(name="rsb", bufs=2))
            npool = ctx.enter_context(tc.tile_pool(name="ytn", bufs=3))
            opool = ctx.enter_context(tc.tile_pool(name="ostage", bufs=6))
            # PSUM: 8 banks total
            pjps = ctx.enter_context(
                tc.tile_pool(name="pj", bufs=3, space="PSUM"))
            stps = ctx.enter_context(
                tc.tile_pool(name="stm", bufs=2, space="PSUM"))
            yhps = ctx.enter_context(
                tc.tile_pool(name="yh", bufs=2, space="PSUM"))
            wops = ctx.enter_context(
                tc.tile_pool(name="wops", bufs=1, space="PSUM"))

            # ---- weights/constants (loaded once; weights FIRST on the Act
            # queue — the first proj matmul gates on wq chunk 0 — while the
            # SP queue streams x) ----
            t_wqs = []
            for i in range(4):
                t = wpool.tile([P, 4, DOUT], F32R, tag=f"wq{i}")
                nc.scalar.dma_start(t[:], wq.ap()[:, i * 4:(i + 1) * 4, :])
                t_wqs.append(t)
            t_wkvs = []
            for i in range(2):
                t = wpool.tile([P, 8, 2 * HD], F32R, tag=f"wkv{i}")
                nc.scalar.dma_start(t[:], wkv.ap()[:, i * 8:(i + 1) * 8, :])
                t_wkvs.append(t)
            t_wo = wpool.tile([P, 2, D], F32R)
            # (t_wo DMA is emitted after batch 0's x loads — Wo(b0) doesn't
            # need it for ~40us and it must not block the x stream)

            t_cc = cpool.tile([P, T], F32)
            nc.scalar.dma_start(t_cc[:], cc.ap())
            t_ss = cpool.tile([P, T], F32)
            nc.scalar.dma_start(t_ss[:], ss.ap())
            t_o64n = cpool.tile([64, 2], F32R)
            nc.scalar.dma_start(t_o64n[:], o64n.ap())
            t_o1x64 = cpool.tile([1, 64], F32R)
            nc.scalar.dma_start(t_o1x64[:], o1x64.ap())
            t_id = cpool.tile([64, 64], F32R)
            nc.scalar.dma_start(t_id[:], ident.ap())
            t_tri = cpool.tile([P, P], BF16)
            nc.scalar.dma_start(t_tri[:], triu.ap())
            t_tri2 = cpool.tile([P, 2 * P], BF16)
            nc.scalar.dma_start(t_tri2[:], triu2.ap())

            def wq_dc(dc):
                return t_wqs[dc // 4][:, dc % 4, :]

            def wkv_dc(dc):
                return t_wkvs[dc // 8][:, dc % 8, :]

            xT3 = xT.ap().rearrange("(c p) t -> p c t", p=P)  # [128, 16, 4096]

            # software-pipelined emission: per-batch state lives in S[b];
            # batch b+1's projection passes are emitted between batch b's
            # attention halves so the rope chain (Act copies + Pool muls)
            # outranks batch b's attention tail and never starves.
            S = {}

            def emit_loads(b):
                tok0 = b * T
                xt = {}
                xdma = {}
                for hf in range(2):
                    for q8 in range(8):
                        t = xpool.tile([P, 2, 512], F32R, tag="xh")
                        # first-batch fill: spread half-A loads over 2 queues
                        eng = nc.gpsimd if (b == 0 and hf == 0 and q8 % 2) \
                            else nc.sync
                        xdma[(hf, q8)] = eng.dma_start(
                            t[:], xT3[:, q8 * 2:(q8 + 1) * 2,
                                      tok0 + hf * 512: tok0 + (hf + 1) * 512])
                        xt[(hf, q8)] = t
                        if b > 0:
                            # pace the stream one batch behind: keeps b's
                            # loads off the DMA fabric while batch b-1's
                            # tail chunks are still streaming
                            add_dep_helper(
                                xdma[(hf, q8)].ins,
                                S[b - 1]["xdma"][(hf, q8)].ins,
                                reason="x stream pacing")
                # per-batch persistent SBUF tiles; q-heads stored in pairs
                # so the 128-wide diagonal score pieces can pack 2 heads
                # into one >=256-wide matmul (fp32r full rate)
                vhat = vpool.tile([P, NTC, HD + 1], BF16, tag="vhat")
                nc.sync.dma_start(vhat[:, :, HD:HD + 1], onestc.ap())
                S[b] = dict(
                    xt=xt, xdma=xdma,
                    qsb=[qpool.tile([64, 2, T], F32R, tag="qhat",
                                    name=f"qsb_{b}_{p}") for p in range(2)],
                    khat=kpool.tile([64, T], F32R, tag="khat",
                                    name=f"khat_{b}"),
                    vhat=vhat,
                    kbias=bpool.tile([P, NTC], F32, tag="kbias",
                                     name=f"kbias_{b}"),
                    ytn=[npool.tile([P, T], F32R, tag="ytn",
                                    name=f"ytn_{b}_{i}") for i in range(2)])

            def rope_q(b, acc, h0, hf):
                """acc [128,512] PSUM holds heads (h0, h0+1); half hf."""
                s = slice(hf * 512, (hf + 1) * 512)
                raw = rawp.tile([P, 512], F32, tag="raw")
                nc.scalar.copy(raw[:], acc[:])
                t1 = tpool.tile([P, 512], F32, tag="t1")
                nc.gpsimd.tensor_mul(t1[:], raw[:], t_cc[:, s])
                t2 = tpool.tile([P, 512], F32, tag="t2")
                for bp in (0, 64):
                    nc.gpsimd.tensor_mul(
                        t2[bp:bp + 32, :], raw[bp + 32:bp + 64, :],
                        t_ss[bp + 32:bp + 64, s])
                    nc.gpsimd.tensor_mul(
                        t2[bp + 32:bp + 64, :], raw[bp:bp + 32, :],
                        t_ss[bp:bp + 32, s])
                for i in range(2):
                    bp = i * 64
                    nc.gpsimd.tensor_add(
                        S[b]["qsb"][h0 // 2][:, i, s], t1[bp:bp + 64, :],
                        t2[bp:bp + 64, :])

            def rope_kv(b, acc, hf):
                """acc [128,512] PSUM: rows 0:64 K, 64:128 V; half hf."""
                s = slice(hf * 512, (hf + 1) * 512)
                khat, vhat, kbias = S[b]["khat"], S[b]["vhat"], S[b]["kbias"]
                ksb = rawp.tile([64, 512], F32, tag="raw")
                nc.scalar.copy(ksb[:], acc[0:64, :])
                vsb = rawp.tile([64, 512], F32R, tag="vsb")
                nc.scalar.copy(vsb[:], acc[64:128, :])
                t1 = tpool.tile([64, 512], F32, tag="t1")
                nc.gpsimd.tensor_mul(t1[:], ksb[0:64, :], t_cc[0:64, s])
                t2 = tpool.tile([64, 512], F32, tag="t2")
                nc.gpsimd.tensor_mul(
                    t2[0:32, :], ksb[32:64, :], t_ss[32:64, s])
                nc.gpsimd.tensor_mul(
                    t2[32:64, :], ksb[0:32, :], t_ss[0:32, s])
                nc.gpsimd.tensor_add(khat[:, s], t1[:], t2[:])
                # k-norm bias from PRE-rope K (rotation preserves norms):
                # 4 two-col matmuls -> [128,8] psum -> kbias. Runs parallel
                # with the rope arithmetic instead of after it.
                k2 = tpool.tile([64, 512], F32R, tag="t1")
                nc.gpsimd.tensor_mul(k2[:], ksb[0:64, :], ksb[0:64, :])
                kbp = stps.tile([P, 8], F32, tag="stm")
                for i in range(4):
                    nc.tensor.matmul(
                        kbp[:, 2 * i:2 * i + 2], k2[:, i * P:(i + 1) * P],
                        t_o64n[:], start=True, stop=True)
                nc.vector.tensor_copy(
                    kbias[:, hf * 4:(hf + 1) * 4],
                    kbp[:].rearrange("p (c two) -> p c two", two=2)[:, :, 0])
                # V transposes into vhat
                for i in range(4):
                    kc = hf * 4 + i
                    tp = stps.tile([P, 64], F32R, tag="stm")
                    nc.tensor.transpose(
                        tp[:], vsb[:, i * P:(i + 1) * P], t_id[:])
                    nc.scalar.copy(vhat[:, kc, 0:HD], tp[:].bitcast(F32))

            def emit_pass(b, hf):
                """projection pass for one 512-token half + its ropes."""
                xt = S[b]["xt"]

                def xsrc(dc):
                    return xt[(hf, dc // 2)][:, dc % 2, :]

                acc_q0 = pjps.tile([P, 512], F32, tag="pj")
                acc_q1 = pjps.tile([P, 512], F32, tag="pj")
                acc_kv = pjps.tile([P, 512], F32, tag="pj")
                for dc in range(NDC):
                    st = (dc == 0)
                    sp = (dc == NDC - 1)
                    nc.tensor.matmul(
                        acc_q0[:], wq_dc(dc)[:, 0:P], xsrc(dc),
                        start=st, stop=sp, skip_group_check=True)
                    nc.tensor.matmul(
                        acc_q1[:], wq_dc(dc)[:, P:2 * P], xsrc(dc),
                        start=st, stop=sp, skip_group_check=True)
                    nc.tensor.matmul(
                        acc_kv[:], wkv_dc(dc), xsrc(dc),
                        start=st, stop=sp, skip_group_check=True)
                rope_q(b, acc_q0, 0, hf)
                rope_q(b, acc_q1, 2, hf)
                rope_kv(b, acc_kv, hf)

            def emit_attn_wo(b, c_lo, cw):
                # attention for query cols [c_lo, c_lo+cw) + their Wo
                # chunks. Those cols need only kc 0..(c_lo+cw)/128 (causal),
                # so early column groups depend only on the early ropes and
                # their Wo overlaps the later groups' attention.
                tok0 = b * T
                qsb, khat = S[b]["qsb"], S[b]["khat"]
                vhat, kbias, ytn = S[b]["vhat"], S[b]["kbias"], S[b]["ytn"]
                nkc = (c_lo + cw) // P
                kcd = nkc - 1        # the 128-wide diagonal chunk
                # pack the 128-wide diag pieces: 2 heads per matmul so
                # the moving operand is 256 wide (fp32r full rate)
                ad = kcd * P
                pt4 = []
                for pair in range(2):
                    # fp32r matmul needs a 2D moving AP: stage the two
                    # heads' diag columns contiguously first
                    qd = rpool.tile([64, 256], F32R, tag="qd")
                    nc.gpsimd.tensor_copy(
                        qd[:], qsb[pair][:, :, ad:ad + P])
                    st4 = stps.tile([P, 256], F32, tag="stm")
                    nc.tensor.matmul(
                        st4[:], khat[:, ad:ad + P],
                        qd[:], start=True, stop=True)
                    p4 = ptpool.tile([P, 256], BF16, tag="pt")
                    nc.scalar.activation(
                        p4[:], st4[:], AF.Exp,
                        bias=kbias[:, kcd:kcd + 1], scale=0.125)
                    nc.vector.tensor_mul(p4[:], p4[:], t_tri2[:])
                    pt4.append(p4)
                for h in range(HPC):
                    # drain batch: rotate attention accumulators through the
                    # now-idle 3-slot proj pool for deeper head pipelining
                    yp_ = pjps if b == B - 1 else yhps
                    yh = yp_.tile([65, cw], F32,
                                  tag="pj" if yp_ is pjps else "yh")
                    for kc in range(nkc):
                        q0 = kc * P
                        a0 = max(q0, c_lo)          # abs col start
                        w = c_lo + cw - a0
                        if kc == kcd:
                            pt, off = pt4[h // 2], (h % 2) * P
                        else:
                            off = 0
                            pt = ptpool.tile([P, w], BF16, tag="pt")
                            st = stps.tile([P, w], F32, tag="stm")
                            nc.tensor.matmul(
                                st[:], khat[:, q0:q0 + P],
                                qsb[h // 2][:, h % 2, a0:a0 + w],
                                start=True, stop=True)
                            nc.scalar.activation(
                                pt[:, 0:w], st[:], AF.Exp,
                                bias=kbias[:, kc:kc + 1], scale=0.125)
                            if a0 == q0:  # diagonal block: causal mask
                                nc.vector.tensor_mul(
                                    pt[:, 0:P], pt[:, 0:P], t_tri[:])
                        nc.tensor.matmul(
                            yh[:, a0 - c_lo:a0 - c_lo + w],
                            vhat[:, kc, :], pt[:, off:off + w],
                            start=(kc == 0), stop=(kc == nkc - 1),
                            skip_group_check=True)
                    # normalize: recip straight off PSUM row 64, then a
                    # GpSimd partition-broadcast instead of a PE matmul
                    rsb = rpool.tile([1, cw], F32R, tag="rsb")
                    with nc.allow_low_precision(reason="f32r recip row"):
                        nc.vector.reciprocal(rsb[:], yh[64:65, :])
                    ysb = ypool.tile([64, cw], F32, tag="ysb")
                    nc.scalar.copy(ysb[:], yh[0:64, :])
                    rb = rpool.tile([64, cw], F32R, tag="rb")
                    nc.gpsimd.partition_broadcast(rb[:], rsb[:], channels=64)
                    bp = (h % 2) * 64
                    nc.vector.tensor_mul(
                        ytn[h // 2][bp:bp + 64, c_lo:c_lo + cw],
                        ysb[0:64, :], rb[:].bitcast(F32))

                # Wo for this column group's token chunks
                for tcn in range(c_lo // P, (c_lo + cw) // P):
                    for oq in range(4):
                        ci = tcn * 4 + oq
                        # last batch: borrow the idle proj pool so the
                        # drain isn't serialized on one PSUM bank
                        # (drain batch: yh pool is idle — rotate Wo through it)
                        pool_ = wops if (b < B - 1 or ci % 3 == 0) else yhps
                        ops_ = pool_.tile([P, 512], F32,
                                          tag="wops" if pool_ is wops else "yh")
                        for hc in range(2):
                            nc.tensor.matmul(
                                ops_[:], ytn[hc][:, tcn * P:(tcn + 1) * P],
                                t_wo[:, hc, oq * 512:(oq + 1) * 512],
                                start=(hc == 0), stop=(hc == 1))
                        ostg = opool.tile([P, 512], BF16, tag="ostage")
                        # drain batch is DVE-bound (masks+recip+ytn): stage
                        # most of its Wo output on the half-idle Act engine
                        if b == B - 1 and ci % 2 != 0:
                            nc.scalar.copy(ostg[:], ops_[:])
                        else:
                            nc.vector.tensor_copy(ostg[:], ops_[:])
                        eng = nc.sync if ci % 3 == 0 else nc.gpsimd
                        eng.dma_start(
                            out.ap()[tok0 + tcn * P: tok0 + (tcn + 1) * P,
                                     oq * 512:(oq + 1) * 512],
                            ostg[:])

            # ---- pipelined emission order ----
            emit_loads(0)
            # hold the 8MB t_wo load off the DMA fabric until batch 0's
            # critical half-A x stream is through (Wo(b0) runs ~40us in)
            wo_dma = nc.gpsimd.dma_start(t_wo[:], wo.ap())
            add_dep_helper(wo_dma.ins, S[0]["xdma"][(1, 0)].ins,
                           reason="t_wo load after b0 half-A x stream")
            emit_pass(0, 0)
            emit_pass(0, 1)
            for b in range(B):
                if b + 1 < B:
                    emit_loads(b + 1)
                emit_attn_wo(b, 0, 512)
                if b + 1 < B:
                    emit_pass(b + 1, 0)
                emit_attn_wo(b, 512, 512)
                if b + 1 < B:
                    emit_pass(b + 1, 1)

    nc.finalize()
    return nc


def _host_consts():
    inv = 1.0 / (ROPE_BASE ** (np.arange(0, HD, 2, dtype=np.float32) / HD))
    ang = np.arange(T, dtype=np.float32)[:, None] * inv[None, :]  # [T, 32]
    cosr = np.cos(ang).T.astype(np.float32)                        # [32, T]
    sinr = np.sin(ang).T.astype(np.float32)
    cc = np.tile(cosr, (4, 1))                                     # [128, T]
    # signed sin table: +sin on x1 rows (j<32), -sin on x2 rows (j>=32);
    # reading row r of ssx multiplies the operand that LANDS shifted by +-32.
    ss = np.tile(np.concatenate([sinr, -sinr], axis=0), (2, 1))
    consts = {
        "cc": np.ascontiguousarray(cc),
        "ss": np.ascontiguousarray(ss),
        "o64n": np.full((64, 2), -0.0625, np.float32),
        "o1x64": np.ones((1, 64), np.float32),
        "ident": np.eye(64, dtype=np.float32),
        "triu": np.triu(np.ones((P, P), np.float32)).astype(_BF),
        "triu2": np.tile(np.triu(np.ones((P, P), np.float32)),
                         (1, 2)).astype(_BF),
        "onestc": np.ones((P, NTC), np.float32).astype(_BF),
    }
    return consts


def kernel(x, Wq, Wk, Wv, Wo):
    x = np.asarray(x, np.float32)
    Wq = np.asarray(Wq, np.float32)
    Wk = np.asarray(Wk, np.float32)
    Wv = np.asarray(Wv, np.float32)
    Wo = np.asarray(Wo, np.float32)
    b, t, d = x.shape

    key = "nc"
    if key not in _COMPILED:
        _COMPILED[key] = _build_nc()
    nc = _COMPILED[key]

    xTh = np.ascontiguousarray(x.reshape(b * t, d).T)  # [2048, 4096]
    consts = _host_consts()

    in_maps = []
    for c in range(NCORES):
        wq_c = np.ascontiguousarray(
            Wq[:, c * DOUT:(c + 1) * DOUT].reshape(NDC, P, DOUT)
            .transpose(1, 0, 2))
        wkv_np = np.concatenate(
            [Wk[:, c * HD:(c + 1) * HD], Wv[:, c * HD:(c + 1) * HD]], axis=1)
        wkv_c = np.ascontiguousarray(
            wkv_np.reshape(NDC, P, 2 * HD).transpose(1, 0, 2))
        wo_c = np.ascontiguousarray(
            Wo[c * DOUT:(c + 1) * DOUT, :].reshape(2, P, d).transpose(1, 0, 2))
        m = {"xT": xTh, "wq": wq_c, "wkv": wkv_c, "wo": wo_c}
        m.update(consts)
        in_maps.append(m)

    res = run_bass_kernel_spmd(nc, in_maps, list(range(NCORES)))
    acc = res.results[0]["out"].astype(np.float32)
    for c in range(1, NCORES):
        acc = acc + res.results[c]["out"].astype(np.float32)
    return acc.reshape(b, t, d)


if __name__ == "__main__":
    rng = np.random.default_rng(0)
    x = rng.standard_normal((B, T, D), dtype=np.float32)
    Wq = (rng.standard_normal((D, D), dtype=np.float32) * 0.02)
    Wk = (rng.standard_normal((D, KV), dtype=np.float32) * 0.02)
    Wv = (rng.standard_normal((D, KV), dtype=np.float32) * 0.02)
    Wo = (rng.standard_normal((D, D), dtype=np.float32) * 0.02)
    y = kernel(x=x, Wq=Wq, Wk=Wk, Wv=Wv, Wo=Wo)
    print("out", y.shape, y.dtype, np.abs(y).max())


# revision 114
# speedup vs baseline: 1.0081x; 1.0081x over previous
"""Trainium2 Bass kernel for ConformalGQA (b=4, t=1024, d=2048, 32 Q heads /
8 KV heads, hd=64, RoPE, causal, scores = (q.k - |q|^2/2 - |k|^2/2)/sqrt(hd)).

Sharding: 8-way tensor-parallel over heads. Core c owns Q heads 4c..4c+3 and
KV head c (Wq/Wk/Wv column-sharded, Wo row-sharded). Each core emits a full
(4096, 2048) partial output; the host sums the 8 partials.

Key device-side structure (v2 — engine-balanced, PSUM-budgeted pipeline):
 - The -0.5|q|^2 term of the scores is constant per query row, so softmax
   cancels it: it is DROPPED entirely. Only the k-norm survives, folded into
   the per-partition bias of the Exp activation (bias = -|k|^2/16). Scores
   are then bounded by |q|^2/16 (~3, max ~10): exp stays well inside fp32.
 - Projections run in half-token passes: per 512-token half, three [128,512]
   PSUM accumulators (q-heads 0:2, q-heads 2:4, K|V) accumulate over the 16
   d-chunks. 1 PSUM bank each; rope consumes them via an Act-engine copy to
   SBUF, so PE never waits on DVE.
 - Scores are computed as S^T (k on partitions, q on free dim) in >=256-wide
   pieces (remainder-free splits) to keep fp32r at 1 cycle/row.
 - P^T = exp(S^T/8 + bias) -> attnV accumulates [V | 1]^T @ P^T so the
   softmax denominator rides in row 64 of yhp for free.
 - PSUM budget (8 banks): proj pool 3x1, scores/misc pool 2x1, yhp 1x2,
   Wo out pool 1x1.
 - Engine split: PE matmuls; Act: exp + PSUM->SBUF staging copies; DVE:
   rope arithmetic + normalize; Pool(GpSimd): causal diag masks + out-DMA
   queue; SP: x-in DMA queue.
All matmuls run in float32r (single-pass fp32 PE mode).
"""

import sys

for _p in ("/opt/trn_rl_repo",):
    if _p not in sys.path:
        sys.path.insert(0, _p)

import numpy as np
import ml_dtypes
from contextlib import ExitStack

_BF = ml_dtypes.bfloat16

import concourse.bass as bass
import concourse.mybir as mybir
import concourse.tile as tile
from concourse.tile import add_dep_helper
from concourse import bacc
from concourse.bass_utils import run_bass_kernel_spmd

F32R = mybir.dt.float32r
F32 = mybir.dt.float32
BF16 = mybir.dt.bfloat16
AF = mybir.ActivationFunctionType

B, T, D, KV = 4, 1024, 2048, 512
H, HKV, HD = 32, 8, 64
P = 128
NCORES = 8
HPC = H // NCORES          # 4 q heads per core
DOUT = HPC * HD            # 256 q-proj cols per core
NDC = D // P               # 16 contraction chunks
NTC = T // P               # 8 key chunks per batch
ROPE_BASE = 10000.0

_COMPILED = {}


def _score_pieces(kc):
    """Column pieces [c0, c0+cw) covering queries [q0, T) with cw >= 256
    where possible (each piece is one matmul into its own 1-bank tile)."""
    q0 = kc * P
    w = T - q0
    if w <= 512:
        return [(q0, w)]
    if w - 512 >= 256:
        return [(q0, 512), (q0 + 512, w - 512)]
    return [(q0, w - 256), (q0 + w - 256, 256)]


def _attnv_pieces(kc):
    """Accumulation pieces into yhp [65, T]: must not cross the tile's
    internal PSUM bank boundary at column 512."""
    q0 = kc * P
    if q0 < 512:
        return [(q0, 512 - q0), (512, 512)]
    return [(q0, T - q0)]


def _build_nc():
    nc = bacc.Bacc("TRN2", target_bir_lowering=False, debug=False,
                   num_devices=NCORES)

    xT = nc.dram_tensor("xT", [D, B * T], F32R, kind="ExternalInput")
    wq = nc.dram_tensor("wq", [P, NDC, DOUT], F32R, kind="ExternalInput")
    wkv = nc.dram_tensor("wkv", [P, NDC, 2 * HD], F32R, kind="ExternalInput")
    wo = nc.dram_tensor("wo", [P, 2, D], F32R, kind="ExternalInput")
    cc = nc.dram_tensor("cc", [P, T], F32, kind="ExternalInput")
    ss = nc.dram_tensor("ss", [P, T], F32, kind="ExternalInput")
    o64n = nc.dram_tensor("o64n", [64, 2], F32R, kind="ExternalInput")
    o1x64 = nc.dram_tensor("o1x64", [1, 64], F32R, kind="ExternalInput")
    ident = nc.dram_tensor("ident", [64, 64], F32R, kind="ExternalInput")
    triu = nc.dram_tensor("triu", [P, P], BF16, kind="ExternalInput")
    triu2 = nc.dram_tensor("triu2", [P, 2 * P], BF16, kind="ExternalInput")
    onestc = nc.dram_tensor("onestc", [P, NTC], BF16, kind="ExternalInput")
    out = nc.dram_tensor("out", [B * T, D], BF16, kind="ExternalOutput")

    with tile.TileContext(nc) as tc:
        with ExitStack() as ctx:
            cpool = ctx.enter_context(tc.tile_pool(name="consts", bufs=1))
            wpool = ctx.enter_context(tc.tile_pool(name="weights", bufs=1))
            xpool = ctx.enter_context(tc.tile_pool(name="x", bufs=10))
            rawp = ctx.enter_context(tc.tile_pool(name="raw", bufs=4))
            tpool = ctx.enter_context(tc.tile_pool(name="tscr", bufs=4))
            qpool = ctx.enter_context(tc.tile_pool(name="qhat", bufs=4))
            kpool = ctx.enter_context(tc.tile_pool(name="khat", bufs=2))
            vpool = ctx.enter_context(tc.tile_pool(name="vhat", bufs=2))
            bpool = ctx.enter_context(tc.tile_pool(name="kbias", bufs=2))
            ptpool = ctx.enter_context(tc.tile_pool(name="pt", bufs=7))
            rpool = ctx.enter_context(tc.tile_pool(name="rsb", bufs=2))
            npool = ctx.enter_context(tc.tile_pool(name="ytn", bufs=4))
            opool = ctx.enter_context(tc.tile_pool(name="ostage", bufs=6))
            # PSUM: 8 banks total
            pjps = ctx.enter_context(
                tc.tile_pool(name="pj", bufs=3, space="PSUM"))
            stps = ctx.enter_context(
                tc.tile_pool(name="stm", bufs=2, space="PSUM"))
            yhps = ctx.enter_context(
                tc.tile_pool(name="yh", bufs=2, space="PSUM"))
            wops = ctx.enter_context(
                tc.tile_pool(name="wops", bufs=1, space="PSUM"))

            # ---- weights/constants (loaded once; weights FIRST on the Act
            # queue — the first proj matmul gates on wq chunk 0 — while the
            # SP queue streams x) ----
            t_wqs = []
            for i in range(4):
                t = wpool.tile([P, 4, DOUT], F32R, tag=f"wq{i}")
                nc.scalar.dma_start(t[:], wq.ap()[:, i * 4:(i + 1) * 4, :])
                t_wqs.append(t)
            t_wkvs = []
            for i in range(2):
                t = wpool.tile([P, 8, 2 * HD], F32R, tag=f"wkv{i}")
                nc.scalar.dma_start(t[:], wkv.ap()[:, i * 8:(i + 1) * 8, :])
                t_wkvs.append(t)
            t_wo = wpool.tile([P, 2, D], F32R)
            # (t_wo DMA is emitted after batch 0's x loads — Wo(b0) doesn't
            # need it for ~40us and it must not block the x stream)

            t_cc = cpool.tile([P, T], F32)
            nc.scalar.dma_start(t_cc[:], cc.ap())
            t_ss = cpool.tile([P, T], F32)
            nc.scalar.dma_start(t_ss[:], ss.ap())
            t_o64n = cpool.tile([64, 2], F32R)
            nc.scalar.dma_start(t_o64n[:], o64n.ap())
            t_o1x64 = cpool.tile([1, 64], F32R)
            nc.scalar.dma_start(t_o1x64[:], o1x64.ap())
            t_id = cpool.tile([64, 64], F32R)
            nc.scalar.dma_start(t_id[:], ident.ap())
            t_tri = cpool.tile([P, P], BF16)
            nc.scalar.dma_start(t_tri[:], triu.ap())
            t_tri2 = cpool.tile([P, 2 * P], BF16)
            nc.scalar.dma_start(t_tri2[:], triu2.ap())

            def wq_dc(dc):
                return t_wqs[dc // 4][:, dc % 4, :]

            def wkv_dc(dc):
                return t_wkvs[dc // 8][:, dc % 8, :]

            xT3 = xT.ap().rearrange("(c p) t -> p c t", p=P)  # [128, 16, 4096]

            # software-pipelined emission: per-batch state lives in S[b];
            # batch b+1's projection passes are emitted between batch b's
            # attention halves so the rope chain (Act copies + Pool muls)
            # outranks batch b's attention tail and never starves.
            S = {}

            def emit_loads(b):
                tok0 = b * T
                xt = {}
                xdma = {}
                for hf in range(2):
                    for q8 in range(8):
                        t = xpool.tile([P, 2, 512], F32R, tag="xh")
                        # first-batch fill: spread half-A loads over 2 queues
                        eng = nc.gpsimd if (b == 0 and hf == 0 and q8 % 2) \
                            else nc.sync
                        xdma[(hf, q8)] = eng.dma_start(
                            t[:], xT3[:, q8 * 2:(q8 + 1) * 2,
                                      tok0 + hf * 512: tok0 + (hf + 1) * 512])
                        xt[(hf, q8)] = t
                        if b > 0:
                            # pace the stream one batch behind: keeps b's
                            # loads off the DMA fabric while batch b-1's
                            # tail chunks are still streaming
                            add_dep_helper(
                                xdma[(hf, q8)].ins,
                                S[b - 1]["xdma"][(hf, q8)].ins,
                                reason="x stream pacing")
                # per-batch persistent SBUF tiles; q-heads stored in pairs
                # so the 128-wide diagonal score pieces can pack 2 heads
                # into one >=256-wide matmul (fp32r full rate)
                vhat = vpool.tile([P, NTC, HD + 1], BF16, tag="vhat")
                nc.sync.dma_start(vhat[:, :, HD:HD + 1], onestc.ap())
                S[b] = dict(
                    xt=xt, xdma=xdma,
                    qsb=[qpool.tile([64, 2, T], F32R, tag="qhat",
                                    name=f"qsb_{b}_{p}") for p in range(2)],
                    khat=kpool.tile([64, T], F32R, tag="khat",
                                    name=f"khat_{b}"),
                    vhat=vhat,
                    kbias=bpool.tile([P, NTC], F32, tag="kbias",
                                     name=f"kbias_{b}"),
                    ytn=[npool.tile([P, T], F32R, tag="ytn",
                                    name=f"ytn_{b}_{i}") for i in range(2)])

            def rope_q(b, acc, h0, hf):
                """acc [128,512] PSUM holds heads (h0, h0+1); half hf."""
                s = slice(hf * 512, (hf + 1) * 512)
                raw = rawp.tile([P, 512], F32, tag="raw")
                nc.scalar.copy(raw[:], acc[:])
                t1 = tpool.tile([P, 512], F32, tag="t1")
                nc.gpsimd.tensor_mul(t1[:], raw[:], t_cc[:, s])
                t2 = tpool.tile([P, 512], F32, tag="t2")
                for bp in (0, 64):
                    nc.gpsimd.tensor_mul(
                        t2[bp:bp + 32, :], raw[bp + 32:bp + 64, :],
                        t_ss[bp + 32:bp + 64, s])
                    nc.gpsimd.tensor_mul(
                        t2[bp + 32:bp + 64, :], raw[bp:bp + 32, :],
                        t_ss[bp:bp + 32, s])
                for i in range(2):
                    bp = i * 64
                    nc.gpsimd.tensor_add(
                        S[b]["qsb"][h0 // 2][:, i, s], t1[bp:bp + 64, :],
                        t2[bp:bp + 64, :])

            def rope_kv(b, acc, hf):
                """acc [128,512] PSUM: rows 0:64 K, 64:128 V; half hf."""
                s = slice(hf * 512, (hf + 1) * 512)
                khat, vhat, kbias = S[b]["khat"], S[b]["vhat"], S[b]["kbias"]
                ksb = rawp.tile([64, 512], F32, tag="raw")
                nc.scalar.copy(ksb[:], acc[0:64, :])
                vsb = rawp.tile([64, 512], F32R, tag="vsb")
                nc.scalar.copy(vsb[:], acc[64:128, :])
                t1 = tpool.tile([64, 512], F32, tag="t1")
                nc.gpsimd.tensor_mul(t1[:], ksb[0:64, :], t_cc[0:64, s])
                t2 = tpool.tile([64, 512], F32, tag="t2")
                nc.gpsimd.tensor_mul(
                    t2[0:32, :], ksb[32:64, :], t_ss[32:64, s])
                nc.gpsimd.tensor_mul(
                    t2[32:64, :], ksb[0:32, :], t_ss[0:32, s])
                nc.gpsimd.tensor_add(khat[:, s], t1[:], t2[:])
                # k-norm bias from PRE-rope K (rotation preserves norms):
                # 4 two-col matmuls -> [128,8] psum -> kbias. Runs parallel
                # with the rope arithmetic instead of after it.
                k2 = tpool.tile([64, 512], F32R, tag="t1")
                nc.gpsimd.tensor_mul(k2[:], ksb[0:64, :], ksb[0:64, :])
                kbp = stps.tile([P, 8], F32, tag="stm")
                for i in range(4):
                    nc.tensor.matmul(
                        kbp[:, 2 * i:2 * i + 2], k2[:, i * P:(i + 1) * P],
                        t_o64n[:], start=True, stop=True)
                nc.vector.tensor_copy(
                    kbias[:, hf * 4:(hf + 1) * 4],
                    kbp[:].rearrange("p (c two) -> p c two", two=2)[:, :, 0])
                # V transposes into vhat
                for i in range(4):
                    kc = hf * 4 + i
                    tp = stps.tile([P, 64], F32R, tag="stm")
                    nc.tensor.transpose(
                        tp[:], vsb[:, i * P:(i + 1) * P], t_id[:])
                    nc.scalar.copy(vhat[:, kc, 0:HD], tp[:].bitcast(F32))

            def emit_pass(b, hf):
                """projection pass for one 512-token half + its ropes."""
                xt = S[b]["xt"]

                def xsrc(dc):
                    return xt[(hf, dc // 2)][:, dc % 2, :]

                acc_q0 = pjps.tile([P, 512], F32, tag="pj")
                acc_q1 = pjps.tile([P, 512], F32, tag="pj")
                acc_kv = pjps.tile([P, 512], F32, tag="pj")
                for dc in range(NDC):
                    st = (dc == 0)
                    sp = (dc == NDC - 1)
                    nc.tensor.matmul(
                        acc_q0[:], wq_dc(dc)[:, 0:P], xsrc(dc),
                        start=st, stop=sp, skip_group_check=True)
                    nc.tensor.matmul(
                        acc_q1[:], wq_dc(dc)[:, P:2 * P], xsrc(dc),
                        start=st, stop=sp, skip_group_check=True)
                    nc.tensor.matmul(
                        acc_kv[:], wkv_dc(dc), xsrc(dc),
                        start=st, stop=sp, skip_group_check=True)
                rope_q(b, acc_q0, 0, hf)
                rope_q(b, acc_q1, 2, hf)
                rope_kv(b, acc_kv, hf)

            def emit_attn_wo(b, c_lo, cw):
                # attention for query cols [c_lo, c_lo+cw) + their Wo
                # chunks. Those cols need only kc 0..(c_lo+cw)/128 (causal),
                # so early column groups depend only on the early ropes and
                # their Wo overlaps the later groups' attention.
                tok0 = b * T
                qsb, khat = S[b]["qsb"], S[b]["khat"]
                vhat, kbias, ytn = S[b]["vhat"], S[b]["kbias"], S[b]["ytn"]
                nkc = (c_lo + cw) // P
                kcd = nkc - 1        # the 128-wide diagonal chunk
                # pack the 128-wide diag pieces: 2 heads per matmul so
                # the moving operand is 256 wide (fp32r full rate)
                ad = kcd * P
                pt4 = []
                for pair in range(2):
                    # fp32r matmul needs a 2D moving AP: stage the two
                    # heads' diag columns contiguously first
                    qd = rpool.tile([64, 256], F32R, tag="qd")
                    nc.gpsimd.tensor_copy(
                        qd[:], qsb[pair][:, :, ad:ad + P])
                    st4 = stps.tile([P, 256], F32, tag="stm")
                    nc.tensor.matmul(
                        st4[:], khat[:, ad:ad + P],
                        qd[:], start=True, stop=True)
                    p4 = ptpool.tile([P, 256], BF16, tag="pt")
                    nc.scalar.activation(
                        p4[:], st4[:], AF.Exp,
                        bias=kbias[:, kcd:kcd + 1], scale=0.125)
                    nc.vector.tensor_mul(p4[:], p4[:], t_tri2[:])
                    pt4.append(p4)
                for h in range(HPC):
                    # drain batch: rotate attention accumulators through the
                    # now-idle 3-slot proj pool for deeper head pipelining
                    yp_ = pjps if b == B - 1 else yhps
                    yh = yp_.tile([65, cw], F32,
                                  tag="pj" if yp_ is pjps else "yh")
                    for kc in range(nkc):
                        q0 = kc * P
                        a0 = max(q0, c_lo)          # abs col start
                        w = c_lo + cw - a0
                        if kc == kcd:
                            pt, off = pt4[h // 2], (h % 2) * P
                        else:
                            off = 0
                            pt = ptpool.tile([P, w], BF16, tag="pt")
                            st = stps.tile([P, w], F32, tag="stm")
                            nc.tensor.matmul(
                                st[:], khat[:, q0:q0 + P],
                                qsb[h // 2][:, h % 2, a0:a0 + w],
                                start=True, stop=True)
                            nc.scalar.activation(
                                pt[:, 0:w], st[:], AF.Exp,
                                bias=kbias[:, kc:kc + 1], scale=0.125)
                            if a0 == q0:  # diagonal block: causal mask
                                nc.vector.tensor_mul(
                                    pt[:, 0:P], pt[:, 0:P], t_tri[:])
                        nc.tensor.matmul(
                            yh[:, a0 - c_lo:a0 - c_lo + w],
                            vhat[:, kc, :], pt[:, off:off + w],
                            start=(kc == 0), stop=(kc == nkc - 1),
                            skip_group_check=True)
                    # normalize: recip straight off PSUM row 64, then a
                    # GpSimd partition-broadcast instead of a PE matmul
                    rsb = rpool.tile([1, cw], F32R, tag="rsb")
                    with nc.allow_low_precision(reason="f32r recip row"):
                        nc.vector.reciprocal(rsb[:], yh[64:65, :])
                    rb = rpool.tile([64, cw], F32R, tag="rb")
                    nc.gpsimd.partition_broadcast(rb[:], rsb[:], channels=64)
                    bp = (h % 2) * 64
                    # multiply straight off the PSUM accumulator — the yh
                    # slot is held ~1.4us longer but 32 staging copies die
                    nc.vector.tensor_mul(
                        ytn[h // 2][bp:bp + 64, c_lo:c_lo + cw],
                        yh[0:64, :], rb[:].bitcast(F32))

                # Wo for this column group's token chunks
                for tcn in range(c_lo // P, (c_lo + cw) // P):
                    for oq in range(4):
                        ci = tcn * 4 + oq
                        # last batch: borrow the idle proj pool so the
                        # drain isn't serialized on one PSUM bank
                        # (drain batch: yh pool is idle — rotate Wo through it)
                        pool_ = wops if (b < B - 1 or ci % 3 == 0) else yhps
                        ops_ = pool_.tile([P, 512], F32,
                                          tag="wops" if pool_ is wops else "yh")
                        for hc in range(2):
                            nc.tensor.matmul(
                                ops_[:], ytn[hc][:, tcn * P:(tcn + 1) * P],
                                t_wo[:, hc, oq * 512:(oq + 1) * 512],
                                start=(hc == 0), stop=(hc == 1))
                        ostg = opool.tile([P, 512], BF16, tag="ostage")
                        # drain batch is DVE-bound (masks+recip+ytn): stage
                        # most of its Wo output on the half-idle Act engine
                        if b == B - 1 and ci % 2 != 0:
                            nc.scalar.copy(ostg[:], ops_[:])
                        else:
                            nc.vector.tensor_copy(ostg[:], ops_[:])
                        eng = nc.sync if ci % 3 == 0 else nc.gpsimd
                        eng.dma_start(
                            out.ap()[tok0 + tcn * P: tok0 + (tcn + 1) * P,
                                     oq * 512:(oq + 1) * 512],
                            ostg[:])

            # ---- pipelined emission order ----
            emit_loads(0)
            # hold the 8MB t_wo load off the DMA fabric until batch 0's
            # critical half-A x stream is through (Wo(b0) runs ~40us in)
            wo_dma = nc.gpsimd.dma_start(t_wo[:], wo.ap())
            add_dep_helper(wo_dma.ins, S[0]["xdma"][(1, 0)].ins,
                           reason="t_wo load after b0 half-A x stream")
            emit_pass(0, 0)
            emit_pass(0, 1)
            for b in range(B):
                if b + 1 < B:
                    emit_loads(b + 1)
                emit_attn_wo(b, 0, 512)
                if b + 1 < B:
                    emit_pass(b + 1, 0)
                emit_attn_wo(b, 512, 512)
                if b + 1 < B:
                    emit_pass(b + 1, 1)

    nc.finalize()
    return nc


def _host_consts():
    inv = 1.0 / (ROPE_BASE ** (np.arange(0, HD, 2, dtype=np.float32) / HD))
    ang = np.arange(T, dtype=np.float32)[:, None] * inv[None, :]  # [T, 32]
    cosr = np.cos(ang).T.astype(np.float32)                        # [32, T]
    sinr = np.sin(ang).T.astype(np.float32)
    cc = np.tile(cosr, (4, 1))                                     # [128, T]
    # signed sin table: +sin on x1 rows (j<32), -sin on x2 rows (j>=32);
    # reading row r of ssx multiplies the operand that LANDS shifted by +-32.
    ss = np.tile(np.concatenate([sinr, -sinr], axis=0), (2, 1))
    consts = {
        "cc": np.ascontiguousarray(cc),
        "ss": np.ascontiguousarray(ss),
        "o64n": np.full((64, 2), -0.0625, np.float32),
        "o1x64": np.ones((1, 64), np.float32),
        "ident": np.eye(64, dtype=np.float32),
        "triu": np.triu(np.ones((P, P), np.float32)).astype(_BF),
        "triu2": np.tile(np.triu(np.ones((P, P), np.float32)),
                         (1, 2)).astype(_BF),
        "onestc": np.ones((P, NTC), np.float32).astype(_BF),
    }
    return consts


def kernel(x, Wq, Wk, Wv, Wo):
    x = np.asarray(x, np.float32)
    Wq = np.asarray(Wq, np.float32)
    Wk = np.asarray(Wk, np.float32)
    Wv = np.asarray(Wv, np.float32)
    Wo = np.asarray(Wo, np.float32)
    b, t, d = x.shape

    key = "nc"
    if key not in _COMPILED:
        _COMPILED[key] = _build_nc()
    nc = _COMPILED[key]

    xTh = np.ascontiguousarray(x.reshape(b * t, d).T)  # [2048, 4096]
    consts = _host_consts()

    in_maps = []
    for c in range(NCORES):
        wq_c = np.ascontiguousarray(
            Wq[:, c * DOUT:(c + 1) * DOUT].reshape(NDC, P, DOUT)
            .transpose(1, 0, 2))
        wkv_np = np.concatenate(
            [Wk[:, c * HD:(c + 1) * HD], Wv[:, c * HD:(c + 1) * HD]], axis=1)
        wkv_c = np.ascontiguousarray(
            wkv_np.reshape(NDC, P, 2 * HD).transpose(1, 0, 2))
        wo_c = np.ascontiguousarray(
            Wo[c * DOUT:(c + 1) * DOUT, :].reshape(2, P, d).transpose(1, 0, 2))
        m = {"xT": xTh, "wq": wq_c, "wkv": wkv_c, "wo": wo_c}
        m.update(consts)
        in_maps.append(m)

    res = run_bass_kernel_spmd(nc, in_maps, list(range(NCORES)))
    acc = res.results[0]["out"].astype(np.float32)
    for c in range(1, NCORES):
        acc = acc + res.results[c]["out"].astype(np.float32)
    return acc.reshape(b, t, d)


if __name__ == "__main__":
    rng = np.random.default_rng(0)
    x = rng.standard_normal((B, T, D), dtype=np.float32)
    Wq = (rng.standard_normal((D, D), dtype=np.float32) * 0.02)
    Wk = (rng.standard_normal((D, KV), dtype=np.float32) * 0.02)
    Wv = (rng.standard_normal((D, KV), dtype=np.float32) * 0.02)
    Wo = (rng.standard_normal((D, D), dtype=np.float32) * 0.02)
    y = kernel(x=x, Wq=Wq, Wk=Wk, Wv=Wv, Wo=Wo)
    print("out", y.shape, y.dtype, np.abs(y).max())


# revision 115
# speedup vs baseline: 1.0137x; 1.0056x over previous
"""Trainium2 Bass kernel for ConformalGQA (b=4, t=1024, d=2048, 32 Q heads /
8 KV heads, hd=64, RoPE, causal, scores = (q.k - |q|^2/2 - |k|^2/2)/sqrt(hd)).

Sharding: 8-way tensor-parallel over heads. Core c owns Q heads 4c..4c+3 and
KV head c (Wq/Wk/Wv column-sharded, Wo row-sharded). Each core emits a full
(4096, 2048) partial output; the host sums the 8 partials.

Key device-side structure (v2 — engine-balanced, PSUM-budgeted pipeline):
 - The -0.5|q|^2 term of the scores is constant per query row, so softmax
   cancels it: it is DROPPED entirely. Only the k-norm survives, folded into
   the per-partition bias of the Exp activation (bias = -|k|^2/16). Scores
   are then bounded by |q|^2/16 (~3, max ~10): exp stays well inside fp32.
 - Projections run in half-token passes: per 512-token half, three [128,512]
   PSUM accumulators (q-heads 0:2, q-heads 2:4, K|V) accumulate over the 16
   d-chunks. 1 PSUM bank each; rope consumes them via an Act-engine copy to
   SBUF, so PE never waits on DVE.
 - Scores are computed as S^T (k on partitions, q on free dim) in >=256-wide
   pieces (remainder-free splits) to keep fp32r at 1 cycle/row.
 - P^T = exp(S^T/8 + bias) -> attnV accumulates [V | 1]^T @ P^T so the
   softmax denominator rides in row 64 of yhp for free.
 - PSUM budget (8 banks): proj pool 3x1, scores/misc pool 2x1, yhp 1x2,
   Wo out pool 1x1.
 - Engine split: PE matmuls; Act: exp + PSUM->SBUF staging copies; DVE:
   rope arithmetic + normalize; Pool(GpSimd): causal diag masks + out-DMA
   queue; SP: x-in DMA queue.
All matmuls run in float32r (single-pass fp32 PE mode).
"""

import sys

for _p in ("/opt/trn_rl_repo",):
    if _p not in sys.path:
        sys.path.insert(0, _p)

import numpy as np
import ml_dtypes
from contextlib import ExitStack

_BF = ml_dtypes.bfloat16

import concourse.bass as bass
import concourse.mybir as mybir
import concourse.tile as tile
from concourse.tile import add_dep_helper
from concourse import bacc
from concourse.bass_utils import run_bass_kernel_spmd

F32R = mybir.dt.float32r
F32 = mybir.dt.float32
BF16 = mybir.dt.bfloat16
AF = mybir.ActivationFunctionType

B, T, D, KV = 4, 1024, 2048, 512
H, HKV, HD = 32, 8, 64
P = 128
NCORES = 8
HPC = H // NCORES          # 4 q heads per core
DOUT = HPC * HD            # 256 q-proj cols per core
NDC = D // P               # 16 contraction chunks
NTC = T // P               # 8 key chunks per batch
ROPE_BASE = 10000.0

_COMPILED = {}


def _score_pieces(kc):
    """Column pieces [c0, c0+cw) covering queries [q0, T) with cw >= 256
    where possible (each piece is one matmul into its own 1-bank tile)."""
    q0 = kc * P
    w = T - q0
    if w <= 512:
        return [(q0, w)]
    if w - 512 >= 256:
        return [(q0, 512), (q0 + 512, w - 512)]
    return [(q0, w - 256), (q0 + w - 256, 256)]


def _attnv_pieces(kc):
    """Accumulation pieces into yhp [65, T]: must not cross the tile's
    internal PSUM bank boundary at column 512."""
    q0 = kc * P
    if q0 < 512:
        return [(q0, 512 - q0), (512, 512)]
    return [(q0, T - q0)]


def _build_nc():
    nc = bacc.Bacc("TRN2", target_bir_lowering=False, debug=False,
                   num_devices=NCORES)

    xT = nc.dram_tensor("xT", [D, B * T], F32R, kind="ExternalInput")
    wq = nc.dram_tensor("wq", [P, NDC, DOUT], F32R, kind="ExternalInput")
    wkv = nc.dram_tensor("wkv", [P, NDC, 2 * HD], F32R, kind="ExternalInput")
    wo = nc.dram_tensor("wo", [P, 2, D], F32R, kind="ExternalInput")
    cc = nc.dram_tensor("cc", [P, T], F32, kind="ExternalInput")
    ss = nc.dram_tensor("ss", [P, T], F32, kind="ExternalInput")
    o64n = nc.dram_tensor("o64n", [64, 2], F32R, kind="ExternalInput")
    o1x64 = nc.dram_tensor("o1x64", [1, 64], F32R, kind="ExternalInput")
    ident = nc.dram_tensor("ident", [64, 64], F32R, kind="ExternalInput")
    triu = nc.dram_tensor("triu", [P, P], BF16, kind="ExternalInput")
    triu2 = nc.dram_tensor("triu2", [P, 2 * P], BF16, kind="ExternalInput")
    onestc = nc.dram_tensor("onestc", [P, NTC], BF16, kind="ExternalInput")
    out = nc.dram_tensor("out", [B * T, D], BF16, kind="ExternalOutput")

    with tile.TileContext(nc) as tc:
        with ExitStack() as ctx:
            cpool = ctx.enter_context(tc.tile_pool(name="consts", bufs=1))
            wpool = ctx.enter_context(tc.tile_pool(name="weights", bufs=1))
            xpool = ctx.enter_context(tc.tile_pool(name="x", bufs=10))
            rawp = ctx.enter_context(tc.tile_pool(name="raw", bufs=4))
            tpool = ctx.enter_context(tc.tile_pool(name="tscr", bufs=4))
            qpool = ctx.enter_context(tc.tile_pool(name="qhat", bufs=4))
            kpool = ctx.enter_context(tc.tile_pool(name="khat", bufs=2))
            vpool = ctx.enter_context(tc.tile_pool(name="vhat", bufs=2))
            bpool = ctx.enter_context(tc.tile_pool(name="kbias", bufs=2))
            ptpool = ctx.enter_context(tc.tile_pool(name="pt", bufs=8))
            rpool = ctx.enter_context(tc.tile_pool(name="rsb", bufs=2))
            npool = ctx.enter_context(tc.tile_pool(name="ytn", bufs=4))
            opool = ctx.enter_context(tc.tile_pool(name="ostage", bufs=6))
            # PSUM: 8 banks total
            pjps = ctx.enter_context(
                tc.tile_pool(name="pj", bufs=3, space="PSUM"))
            stps = ctx.enter_context(
                tc.tile_pool(name="stm", bufs=2, space="PSUM"))
            yhps = ctx.enter_context(
                tc.tile_pool(name="yh", bufs=2, space="PSUM"))
            wops = ctx.enter_context(
                tc.tile_pool(name="wops", bufs=1, space="PSUM"))

            # ---- weights/constants (loaded once; weights FIRST on the Act
            # queue — the first proj matmul gates on wq chunk 0 — while the
            # SP queue streams x) ----
            t_wqs = []
            for i in range(4):
                t = wpool.tile([P, 4, DOUT], F32R, tag=f"wq{i}")
                nc.scalar.dma_start(t[:], wq.ap()[:, i * 4:(i + 1) * 4, :])
                t_wqs.append(t)
            t_wkvs = []
            for i in range(2):
                t = wpool.tile([P, 8, 2 * HD], F32R, tag=f"wkv{i}")
                nc.scalar.dma_start(t[:], wkv.ap()[:, i * 8:(i + 1) * 8, :])
                t_wkvs.append(t)
            t_wo = wpool.tile([P, 2, D], F32R)
            # (t_wo DMA is emitted after batch 0's x loads — Wo(b0) doesn't
            # need it for ~40us and it must not block the x stream)

            t_cc = cpool.tile([P, T], F32)
            nc.scalar.dma_start(t_cc[:], cc.ap())
            t_ss = cpool.tile([P, T], F32)
            nc.scalar.dma_start(t_ss[:], ss.ap())
            t_o64n = cpool.tile([64, 2], F32R)
            nc.scalar.dma_start(t_o64n[:], o64n.ap())
            t_o1x64 = cpool.tile([1, 64], F32R)
            nc.scalar.dma_start(t_o1x64[:], o1x64.ap())
            t_id = cpool.tile([64, 64], F32R)
            nc.scalar.dma_start(t_id[:], ident.ap())
            t_tri = cpool.tile([P, P], BF16)
            nc.scalar.dma_start(t_tri[:], triu.ap())
            t_tri2 = cpool.tile([P, 2 * P], BF16)
            nc.scalar.dma_start(t_tri2[:], triu2.ap())

            def wq_dc(dc):
                return t_wqs[dc // 4][:, dc % 4, :]

            def wkv_dc(dc):
                return t_wkvs[dc // 8][:, dc % 8, :]

            xT3 = xT.ap().rearrange("(c p) t -> p c t", p=P)  # [128, 16, 4096]

            # software-pipelined emission: per-batch state lives in S[b];
            # batch b+1's projection passes are emitted between batch b's
            # attention halves so the rope chain (Act copies + Pool muls)
            # outranks batch b's attention tail and never starves.
            S = {}

            def emit_loads(b):
                tok0 = b * T
                xt = {}
                xdma = {}
                for hf in range(2):
                    for q8 in range(8):
                        t = xpool.tile([P, 2, 512], F32R, tag="xh")
                        # first-batch fill: spread half-A loads over 2 queues
                        eng = nc.gpsimd if (b == 0 and hf == 0 and q8 % 2) \
                            else nc.sync
                        xdma[(hf, q8)] = eng.dma_start(
                            t[:], xT3[:, q8 * 2:(q8 + 1) * 2,
                                      tok0 + hf * 512: tok0 + (hf + 1) * 512])
                        xt[(hf, q8)] = t
                        if b > 0:
                            # pace the stream one batch behind: keeps b's
                            # loads off the DMA fabric while batch b-1's
                            # tail chunks are still streaming
                            add_dep_helper(
                                xdma[(hf, q8)].ins,
                                S[b - 1]["xdma"][(hf, q8)].ins,
                                reason="x stream pacing")
                # per-batch persistent SBUF tiles; q-heads stored in pairs
                # so the 128-wide diagonal score pieces can pack 2 heads
                # into one >=256-wide matmul (fp32r full rate)
                vhat = vpool.tile([P, NTC, HD + 1], BF16, tag="vhat")
                nc.sync.dma_start(vhat[:, :, HD:HD + 1], onestc.ap())
                S[b] = dict(
                    xt=xt, xdma=xdma,
                    qsb=[qpool.tile([64, 2, T], F32R, tag="qhat",
                                    name=f"qsb_{b}_{p}") for p in range(2)],
                    khat=kpool.tile([64, T], F32R, tag="khat",
                                    name=f"khat_{b}"),
                    vhat=vhat,
                    kbias=bpool.tile([P, NTC], F32, tag="kbias",
                                     name=f"kbias_{b}"),
                    ytn=[npool.tile([P, T], F32R, tag="ytn",
                                    name=f"ytn_{b}_{i}") for i in range(2)])

            def rope_q(b, acc, h0, hf):
                """acc [128,512] PSUM holds heads (h0, h0+1); half hf."""
                s = slice(hf * 512, (hf + 1) * 512)
                raw = rawp.tile([P, 512], F32, tag="raw")
                nc.scalar.copy(raw[:], acc[:])
                t1 = tpool.tile([P, 512], F32, tag="t1")
                nc.gpsimd.tensor_mul(t1[:], raw[:], t_cc[:, s])
                t2 = tpool.tile([P, 512], F32, tag="t2")
                for bp in (0, 64):
                    nc.gpsimd.tensor_mul(
                        t2[bp:bp + 32, :], raw[bp + 32:bp + 64, :],
                        t_ss[bp + 32:bp + 64, s])
                    nc.gpsimd.tensor_mul(
                        t2[bp + 32:bp + 64, :], raw[bp:bp + 32, :],
                        t_ss[bp:bp + 32, s])
                for i in range(2):
                    bp = i * 64
                    nc.gpsimd.tensor_add(
                        S[b]["qsb"][h0 // 2][:, i, s], t1[bp:bp + 64, :],
                        t2[bp:bp + 64, :])

            def rope_kv(b, acc, hf):
                """acc [128,512] PSUM: rows 0:64 K, 64:128 V; half hf."""
                s = slice(hf * 512, (hf + 1) * 512)
                khat, vhat, kbias = S[b]["khat"], S[b]["vhat"], S[b]["kbias"]
                ksb = rawp.tile([64, 512], F32, tag="raw")
                nc.scalar.copy(ksb[:], acc[0:64, :])
                vsb = rawp.tile([64, 512], F32R, tag="vsb")
                nc.scalar.copy(vsb[:], acc[64:128, :])
                t1 = tpool.tile([64, 512], F32, tag="t1")
                nc.gpsimd.tensor_mul(t1[:], ksb[0:64, :], t_cc[0:64, s])
                t2 = tpool.tile([64, 512], F32, tag="t2")
                nc.gpsimd.tensor_mul(
                    t2[0:32, :], ksb[32:64, :], t_ss[32:64, s])
                nc.gpsimd.tensor_mul(
                    t2[32:64, :], ksb[0:32, :], t_ss[0:32, s])
                nc.gpsimd.tensor_add(khat[:, s], t1[:], t2[:])
                # k-norm bias from PRE-rope K (rotation preserves norms):
                # 4 two-col matmuls -> [128,8] psum -> kbias. Runs parallel
                # with the rope arithmetic instead of after it.
                k2 = tpool.tile([64, 512], F32R, tag="t1")
                nc.gpsimd.tensor_mul(k2[:], ksb[0:64, :], ksb[0:64, :])
                kbp = stps.tile([P, 8], F32, tag="stm")
                for i in range(4):
                    nc.tensor.matmul(
                        kbp[:, 2 * i:2 * i + 2], k2[:, i * P:(i + 1) * P],
                        t_o64n[:], start=True, stop=True)
                nc.vector.tensor_copy(
                    kbias[:, hf * 4:(hf + 1) * 4],
                    kbp[:].rearrange("p (c two) -> p c two", two=2)[:, :, 0])
                # V transposes into vhat
                for i in range(4):
                    kc = hf * 4 + i
                    tp = stps.tile([P, 64], F32R, tag="stm")
                    nc.tensor.transpose(
                        tp[:], vsb[:, i * P:(i + 1) * P], t_id[:])
                    nc.scalar.copy(vhat[:, kc, 0:HD], tp[:].bitcast(F32))

            def emit_pass(b, hf):
                """projection pass for one 512-token half + its ropes."""
                xt = S[b]["xt"]

                def xsrc(dc):
                    return xt[(hf, dc // 2)][:, dc % 2, :]

                acc_q0 = pjps.tile([P, 512], F32, tag="pj")
                acc_q1 = pjps.tile([P, 512], F32, tag="pj")
                acc_kv = pjps.tile([P, 512], F32, tag="pj")
                for dc in range(NDC):
                    st = (dc == 0)
                    sp = (dc == NDC - 1)
                    nc.tensor.matmul(
                        acc_q0[:], wq_dc(dc)[:, 0:P], xsrc(dc),
                        start=st, stop=sp, skip_group_check=True)
                    nc.tensor.matmul(
                        acc_q1[:], wq_dc(dc)[:, P:2 * P], xsrc(dc),
                        start=st, stop=sp, skip_group_check=True)
                    nc.tensor.matmul(
                        acc_kv[:], wkv_dc(dc), xsrc(dc),
                        start=st, stop=sp, skip_group_check=True)
                rope_q(b, acc_q0, 0, hf)
                rope_q(b, acc_q1, 2, hf)
                rope_kv(b, acc_kv, hf)

            def emit_attn_wo(b, c_lo, cw):
                # attention for query cols [c_lo, c_lo+cw) + their Wo
                # chunks. Those cols need only kc 0..(c_lo+cw)/128 (causal),
                # so early column groups depend only on the early ropes and
                # their Wo overlaps the later groups' attention.
                tok0 = b * T
                qsb, khat = S[b]["qsb"], S[b]["khat"]
                vhat, kbias, ytn = S[b]["vhat"], S[b]["kbias"], S[b]["ytn"]
                nkc = (c_lo + cw) // P
                kcd = nkc - 1        # the 128-wide diagonal chunk
                # pack the 128-wide diag pieces: 2 heads per matmul so
                # the moving operand is 256 wide (fp32r full rate)
                ad = kcd * P
                pt4 = []
                for pair in range(2):
                    # fp32r matmul needs a 2D moving AP: stage the two
                    # heads' diag columns contiguously first
                    qd = rpool.tile([64, 256], F32R, tag="qd")
                    nc.gpsimd.tensor_copy(
                        qd[:], qsb[pair][:, :, ad:ad + P])
                    st4 = stps.tile([P, 256], F32, tag="stm")
                    nc.tensor.matmul(
                        st4[:], khat[:, ad:ad + P],
                        qd[:], start=True, stop=True)
                    p4 = ptpool.tile([P, 256], BF16, tag="pt")
                    nc.scalar.activation(
                        p4[:], st4[:], AF.Exp,
                        bias=kbias[:, kcd:kcd + 1], scale=0.125)
                    nc.vector.tensor_mul(p4[:], p4[:], t_tri2[:])
                    pt4.append(p4)
                for h in range(HPC):
                    # drain batch: rotate attention accumulators through the
                    # now-idle 3-slot proj pool for deeper head pipelining
                    yp_ = pjps if b == B - 1 else yhps
                    yh = yp_.tile([65, cw], F32,
                                  tag="pj" if yp_ is pjps else "yh")
                    for kc in range(nkc):
                        q0 = kc * P
                        a0 = max(q0, c_lo)          # abs col start
                        w = c_lo + cw - a0
                        if kc == kcd:
                            pt, off = pt4[h // 2], (h % 2) * P
                        else:
                            off = 0
                            pt = ptpool.tile([P, w], BF16, tag="pt")
                            st = stps.tile([P, w], F32, tag="stm")
                            nc.tensor.matmul(
                                st[:], khat[:, q0:q0 + P],
                                qsb[h // 2][:, h % 2, a0:a0 + w],
                                start=True, stop=True)
                            nc.scalar.activation(
                                pt[:, 0:w], st[:], AF.Exp,
                                bias=kbias[:, kc:kc + 1], scale=0.125)
                            if a0 == q0:  # diagonal block: causal mask
                                nc.vector.tensor_mul(
                                    pt[:, 0:P], pt[:, 0:P], t_tri[:])
                        nc.tensor.matmul(
                            yh[:, a0 - c_lo:a0 - c_lo + w],
                            vhat[:, kc, :], pt[:, off:off + w],
                            start=(kc == 0), stop=(kc == nkc - 1),
                            skip_group_check=True)
                    # normalize: recip straight off PSUM row 64, then a
                    # GpSimd partition-broadcast instead of a PE matmul
                    rsb = rpool.tile([1, cw], F32R, tag="rsb")
                    with nc.allow_low_precision(reason="f32r recip row"):
                        nc.vector.reciprocal(rsb[:], yh[64:65, :])
                    rb = rpool.tile([64, cw], F32R, tag="rb")
                    nc.gpsimd.partition_broadcast(rb[:], rsb[:], channels=64)
                    bp = (h % 2) * 64
                    # multiply straight off the PSUM accumulator — the yh
                    # slot is held ~1.4us longer but 32 staging copies die
                    nc.vector.tensor_mul(
                        ytn[h // 2][bp:bp + 64, c_lo:c_lo + cw],
                        yh[0:64, :], rb[:].bitcast(F32))

                # Wo for this column group's token chunks
                for tcn in range(c_lo // P, (c_lo + cw) // P):
                    for oq in range(4):
                        ci = tcn * 4 + oq
                        # last batch: borrow the idle proj pool so the
                        # drain isn't serialized on one PSUM bank
                        # (drain batch: yh pool is idle — rotate Wo through it)
                        pool_ = wops if (b < B - 1 or ci % 3 == 0) else yhps
                        ops_ = pool_.tile([P, 512], F32,
                                          tag="wops" if pool_ is wops else "yh")
                        for hc in range(2):
                            nc.tensor.matmul(
                                ops_[:], ytn[hc][:, tcn * P:(tcn + 1) * P],
                                t_wo[:, hc, oq * 512:(oq + 1) * 512],
                                start=(hc == 0), stop=(hc == 1))
                        ostg = opool.tile([P, 512], BF16, tag="ostage")
                        # drain batch is DVE-bound (masks+recip+ytn): stage
                        # most of its Wo output on the half-idle Act engine
                        if b == B - 1 and ci % 2 != 0:
                            nc.scalar.copy(ostg[:], ops_[:])
                        else:
                            nc.vector.tensor_copy(ostg[:], ops_[:])
                        eng = nc.sync if ci % 3 == 0 else nc.gpsimd
                        eng.dma_start(
                            out.ap()[tok0 + tcn * P: tok0 + (tcn + 1) * P,
                                     oq * 512:(oq + 1) * 512],
                            ostg[:])

            # ---- pipelined emission order ----
            emit_loads(0)
            # hold the 8MB t_wo load off the DMA fabric until batch 0's
            # critical half-A x stream is through (Wo(b0) runs ~40us in)
            wo_dma = nc.gpsimd.dma_start(t_wo[:], wo.ap())
            add_dep_helper(wo_dma.ins, S[0]["xdma"][(1, 0)].ins,
                           reason="t_wo load after b0 half-A x stream")
            emit_pass(0, 0)
            emit_pass(0, 1)
            for b in range(B):
                if b + 1 < B:
                    emit_loads(b + 1)
                emit_attn_wo(b, 0, 512)
                if b + 1 < B:
                    emit_pass(b + 1, 0)
                emit_attn_wo(b, 512, 512)
                if b + 1 < B:
                    emit_pass(b + 1, 1)

    nc.finalize()
    return nc


def _host_consts():
    inv = 1.0 / (ROPE_BASE ** (np.arange(0, HD, 2, dtype=np.float32) / HD))
    ang = np.arange(T, dtype=np.float32)[:, None] * inv[None, :]  # [T, 32]
    cosr = np.cos(ang).T.astype(np.float32)                        # [32, T]
    sinr = np.sin(ang).T.astype(np.float32)
    cc = np.tile(cosr, (4, 1))                                     # [128, T]
    # signed sin table: +sin on x1 rows (j<32), -sin on x2 rows (j>=32);
    # reading row r of ssx multiplies the operand that LANDS shifted by +-32.
    ss = np.tile(np.concatenate([sinr, -sinr], axis=0), (2, 1))
    consts = {
        "cc": np.ascontiguousarray(cc),
        "ss": np.ascontiguousarray(ss),
        "o64n": np.full((64, 2), -0.0625, np.float32),
        "o1x64": np.ones((1, 64), np.float32),
        "ident": np.eye(64, dtype=np.float32),
        "triu": np.triu(np.ones((P, P), np.float32)).astype(_BF),
        "triu2": np.tile(np.triu(np.ones((P, P), np.float32)),
                         (1, 2)).astype(_BF),
        "onestc": np.ones((P, NTC), np.float32).astype(_BF),
    }
    return consts


def kernel(x, Wq, Wk, Wv, Wo):
    x = np.asarray(x, np.float32)
    Wq = np.asarray(Wq, np.float32)
    Wk = np.asarray(Wk, np.float32)
    Wv = np.asarray(Wv, np.float32)
    Wo = np.asarray(Wo, np.float32)
    b, t, d = x.shape

    key = "nc"
    if key not in _COMPILED:
        _COMPILED[key] = _build_nc()
    nc = _COMPILED[key]

    xTh = np.ascontiguousarray(x.reshape(b * t, d).T)  # [2048, 4096]
    consts = _host_consts()

    in_maps = []
    for c in range(NCORES):
        wq_c = np.ascontiguousarray(
            Wq[:, c * DOUT:(c + 1) * DOUT].reshape(NDC, P, DOUT)
            .transpose(1, 0, 2))
        wkv_np = np.concatenate(
            [Wk[:, c * HD:(c + 1) * HD], Wv[:, c * HD:(c + 1) * HD]], axis=1)
        wkv_c = np.ascontiguousarray(
            wkv_np.reshape(NDC, P, 2 * HD).transpose(1, 0, 2))
        wo_c = np.ascontiguousarray(
            Wo[c * DOUT:(c + 1) * DOUT, :].reshape(2, P, d).transpose(1, 0, 2))
        m = {"xT": xTh, "wq": wq_c, "wkv": wkv_c, "wo": wo_c}
        m.update(consts)
        in_maps.append(m)

    res = run_bass_kernel_spmd(nc, in_maps, list(range(NCORES)))
    acc = res.results[0]["out"].astype(np.float32)
    for c in range(1, NCORES):
        acc = acc + res.results[c]["out"].astype(np.float32)
    return acc.reshape(b, t, d)


if __name__ == "__main__":
    rng = np.random.default_rng(0)
    x = rng.standard_normal((B, T, D), dtype=np.float32)
    Wq = (rng.standard_normal((D, D), dtype=np.float32) * 0.02)
    Wk = (rng.standard_normal((D, KV), dtype=np.float32) * 0.02)
    Wv = (rng.standard_normal((D, KV), dtype=np.float32) * 0.02)
    Wo = (rng.standard_normal((D, D), dtype=np.float32) * 0.02)
    y = kernel(x=x, Wq=Wq, Wk=Wk, Wv=Wv, Wo=Wo)
    print("out", y.shape, y.dtype, np.abs(y).max())
